# revision 8
# baseline (speedup 1.0000x reference)
"""Trainium2 Bass kernel for nn_CT_loss (data-parallel over batch, 8 cores).

Math (R is a general 3x3 matrix, not orthogonal):
  u   = A P0 + b0          A = R diag(e), b0 = t - 0.5 R e      (per batch)
  c_a = G_a P0 + g0_a      G = R^T A,     g0 = R^T b0
  v~_ai = alpha q_A + beta q_B + h~      (1/s_a folded into alpha/beta/h~;
           host re-scales per-batch partial sums by |s_a| during the gather)
  d_ai = u_i - c_a v~_ai ;  la = sqrt(sum_i d^2 * m_a)
  loss = sum_a [sum(m_a) >= 3B] sum(la) / max(sum(m), 1)   (host gather)

Device design (v5): pixel-major [128 partitions = (b,g), free = 1024 px].
Measured TRN2 prices: DVE ts[1u] ~480ns, tt[1u] ~685ns, tt[3u] ~1750ns;
ACT term[1u] ~1225ns, square[3u] ~2860ns. GPSIMD unusable (slow ucode +
SBUF port contention with DVE). Split for ~36us/engine:
  - DVE : 21 terms (ts 4x: u/c T0+T2, VB) + all tensor_tensor work
  - ACT : 15 terms (u/c T1, VA), squares, sqrt+accum
DMA: x split per channel and issued first so T0 terms start ~9.5us;
q split per a-pair on the scalar ring; masks last.
Mask sums + gating + normalization on host (exact in f64).
"""
import os
import sys

import numpy as np

for _p in ("/opt/trn_rl_repo",):
    if _p not in sys.path:
        sys.path.insert(0, _p)

import concourse.bass as bass
import concourse.bacc as bacc
import concourse.tile as tile
from concourse import mybir
from concourse.bass_utils import run_bass_kernel_spmd

from ml_dtypes import bfloat16

F32 = mybir.dt.float32
BF16 = mybir.dt.bfloat16
AF = mybir.ActivationFunctionType
OP = mybir.AluOpType

B, HW = 64, 128 * 128
NCORES, BPC, G, F = 8, 8, 16, 1024

# a -> (c1, c2, qchA, qchB)
QCH = {0: (1, 2, 0, 1), 1: (0, 2, 2, 3), 2: (0, 1, 4, 5)}

# constants tile columns
CA = 0     # A[i*3+j] 9         (u coefs)
CB0 = 9    # b0[i] 3            (u bias)
CG = 12    # G[a*3+j] 9         (c coefs)
CG0 = 21   # g0[a] 3            (c bias)
CAL = 24   # alpha~[a*3+i] 9    (v qA coef)
CH = 33    # h~[a*3+i] 9        (v bias)
CBE = 42   # beta~[a*3+i] 9     (v qB coef)
CZ = 51    # 0.0
NCST = 52

_BUILT = None
LAST = None


def _bcast3(ap, n):
    """[128, F] AP -> [128, n, F] with step-0 middle dim."""
    return bass.AP(tensor=ap.tensor, offset=ap.offset,
                   ap=[ap.ap[0], [0, n], *ap.ap[1:]])


def _build_nc():
    nc = bacc.Bacc(None)
    x_d = nc.dram_tensor("x", [BPC, G, 3, F], BF16, kind="ExternalInput")
    q_d = nc.dram_tensor("q", [BPC, G, 6, F], BF16, kind="ExternalInput")
    m_d = nc.dram_tensor("m", [BPC, G, 3, F], BF16, kind="ExternalInput")
    c_d = nc.dram_tensor("cst", [128, NCST], F32, kind="ExternalInput")
    o_d = nc.dram_tensor("out", [128, 3], F32, kind="ExternalOutput")

    with tile.TileContext(nc) as tc:
        with tc.tile_pool(name="main", bufs=1) as pool:
            cst = pool.tile([128, NCST], F32, tag="cst")
            X = pool.tile([128, 3, F], BF16, tag="X")
            Q = pool.tile([128, 6, F], BF16, tag="Q")
            M = pool.tile([128, 3, F], BF16, tag="M")

            # DMA order matters: cst (tiny) + x per channel first on sync
            # (T0/T2 terms need only x0/x2); q per a-pair on scalar ring;
            # masks last (needed ~30us in).
            nc.sync.dma_start(cst[:], c_d[:])
            xr = x_d[:].rearrange("b g c f -> (b g) c f")
            for ch in range(3):
                nc.sync.dma_start(X[:, ch, :], xr[:, ch, :])
            qr = q_d[:].rearrange("b g c f -> (b g) c f")
            for a in range(3):
                nc.scalar.dma_start(Q[:, 2 * a:2 * a + 2, :],
                                    qr[:, 2 * a:2 * a + 2, :])
            nc.sync.dma_start(M[:], m_d[:].rearrange("b g c f -> (b g) c f"))

            def cs(j):
                return cst[:, j:j + 1]

            # warm the sqrt_and_others table (identity/square/sqrt co-reside)
            warm = pool.tile([128, 1], BF16, tag="warm")
            nc.scalar.activation(warm[:], cs(CZ), AF.Sqrt)

            acc = pool.tile([128, 3], F32, tag="acc")

            # term helper: out = coef*src + bias on the given engine
            def term(eng, out, src, coef, bias):
                if eng == "act":
                    nc.scalar.activation(out, src, AF.Identity,
                                         bias=bias if bias is not None else cs(CZ),
                                         scale=coef)
                else:
                    if bias is None:
                        nc.vector.tensor_scalar(out, src, coef, None,
                                                op0=OP.mult)
                    else:
                        nc.vector.tensor_scalar(out, src, coef, bias,
                                                op0=OP.mult, op1=OP.add)

            uc = pool.tile([128, 6, F], BF16, tag="uc")   # rows: u0..u2,c0..c2
            tA = pool.tile([128, 6, F], BF16, tag="tA")
            tB = pool.tile([128, 6, F], BF16, tag="tB")

            # T0 terms (x0, with bias): DVE ts; T1 (x1): ACT; T2 (x2): DVE
            for i in range(3):
                term("dve", uc[:, i, :], X[:, 0, :], cs(CA + 3 * i), cs(CB0 + i))
                term("act", tA[:, i, :], X[:, 1, :], cs(CA + 3 * i + 1), None)
                term("dve", tB[:, i, :], X[:, 2, :], cs(CA + 3 * i + 2), None)
            for a in range(3):
                term("dve", uc[:, 3 + a, :], X[:, 0, :], cs(CG + 3 * a), cs(CG0 + a))
                term("act", tA[:, 3 + a, :], X[:, 1, :], cs(CG + 3 * a + 1), None)
                term("dve", tB[:, 3 + a, :], X[:, 2, :], cs(CG + 3 * a + 2), None)
            # packed adds: uc += tA ; uc += tB  (DVE, [6u] each)
            nc.vector.tensor_tensor(uc[:], uc[:], tA[:], op=OP.add)
            nc.vector.tensor_tensor(uc[:], uc[:], tB[:], op=OP.add)
            u = uc[:, 0:3, :]

            v = pool.tile([128, 9, F], BF16, tag="v")
            vb = pool.tile([128, 9, F], BF16, tag="vb")
            t9 = pool.tile([128, 9, F], BF16, tag="t9")
            sq = pool.tile([128, 9, F], BF16, tag="sq")
            la2 = pool.tile([128, 3, F], BF16, tag="la2")
            scr = pool.tile([128, F], BF16, tag="scr")

            # three late VB terms ride ACT (fills its tail dip; DVE is critical)
            vb_eng = ["dve"] * 6 + ["act", "act", "act"]
            for a in range(3):
                _, _, qA, qB = QCH[a]
                for i in range(3):
                    r = 3 * a + i
                    term("act", v[:, r, :], Q[:, qA, :], cs(CAL + r), cs(CH + r))
                    term(vb_eng[r], vb[:, r, :], Q[:, qB, :], cs(CBE + r), None)
                sl = slice(3 * a, 3 * a + 3)
                # v = VA + VB (DVE)
                nc.vector.tensor_tensor(v[:, sl, :], v[:, sl, :],
                                        vb[:, sl, :], op=OP.add)
                # t = c_a * v (DVE, c broadcast over i)
                nc.vector.tensor_tensor(t9[:, sl, :],
                                        _bcast3(uc[:, 3 + a, :], 3),
                                        v[:, sl, :], op=OP.mult)
                # d = u - t (DVE, in place into t9)
                nc.vector.tensor_tensor(t9[:, sl, :], u, t9[:, sl, :],
                                        op=OP.subtract)
                # squares (ACT)
                nc.scalar.activation(sq[:, sl, :], t9[:, sl, :], AF.Square)

            # la2 = sq0 + sq1 + sq2 for all a at once (DVE, strided [3u])
            def sq_i(i):
                return bass.AP(tensor=sq.tensor, offset=i * F,
                               ap=[sq[:].ap[0], [3 * F, 3], [1, F]])
            nc.vector.tensor_tensor(la2[:], sq_i(0), sq_i(1), op=OP.add)
            nc.vector.tensor_tensor(la2[:], la2[:], sq_i(2), op=OP.add)
            # w = la2 * mask (DVE, in place, [3u])
            nc.vector.tensor_tensor(la2[:], la2[:], M[:], op=OP.mult)
            for a in range(3):
                # la = sqrt(w), accumulate along free dim (ACT)
                nc.scalar.activation(scr[:], la2[:, a, :], AF.Sqrt,
                                     accum_out=acc[:, a:a + 1])

            nc.sync.dma_start(o_d[:], acc[:])

    nc.compile()
    return nc


def get_nc():
    global _BUILT
    if _BUILT is None:
        _BUILT = _build_nc()
    return _BUILT


def host_constants(R, T, E):
    """[B, NCST] fp32 constants (fp64 host math) + [B,3] |s| scales."""
    Bn = R.shape[0]
    out = np.zeros((Bn, NCST), np.float64)
    sabs = np.zeros((Bn, 3), np.float64)
    for b in range(Bn):
        Rb = R[b].astype(np.float64)
        tb = T[b].astype(np.float64)
        eb = E[b].astype(np.float64)
        A = Rb * eb[None, :]
        b0 = tb - 0.5 * (Rb @ eb)
        Gm = Rb.T @ A
        g0 = Rb.T @ b0
        s = Rb.T @ tb
        out[b, CA:CA + 9] = A.reshape(-1)
        out[b, CB0:CB0 + 3] = b0
        out[b, CG:CG + 9] = Gm.reshape(-1)
        out[b, CG0:CG0 + 3] = g0
        for a, (c1, c2, _, _) in QCH.items():
            sh = np.sign(s[a]) * max(abs(s[a]), 1e-12) if s[a] != 0 else 1e-12
            sabs[b, a] = abs(s[a])
            h = tb - 0.5 * (A[:, c1] + A[:, c2])
            out[b, CAL + 3 * a:CAL + 3 * a + 3] = A[:, c1] / sh
            out[b, CBE + 3 * a:CBE + 3 * a + 3] = A[:, c2] / sh
            out[b, CH + 3 * a:CH + 3 * a + 3] = h / sh
    return out.astype(np.float32), sabs


def make_in_maps(P0, Q0, M, cst):
    in_maps = []
    for k in range(NCORES):
        sl = slice(k * BPC, (k + 1) * BPC)
        in_maps.append({
            "x": P0[sl].reshape(BPC, 3, G, F).transpose(0, 2, 1, 3).astype(bfloat16),
            "q": Q0[sl].reshape(BPC, 6, G, F).transpose(0, 2, 1, 3).astype(bfloat16),
            "m": M[sl].reshape(BPC, 3, G, F).transpose(0, 2, 1, 3).astype(bfloat16),
            "cst": np.ascontiguousarray(np.repeat(cst[sl], G, axis=0)),
        })
    return in_maps


def kernel(pred_rots, pred_P0, pred_Q0, gt_occmask, roi_extent, pred_transes):
    global LAST
    R = np.asarray(pred_rots, np.float32)
    P0 = np.asarray(pred_P0, np.float32)
    Q0 = np.asarray(pred_Q0, np.float32)
    M = np.asarray(gt_occmask, np.float32)
    E = np.asarray(roi_extent, np.float32)
    T = np.asarray(pred_transes, np.float32)

    nc = get_nc()
    cst, sabs = host_constants(R, T, E)
    in_maps = make_in_maps(P0, Q0, M, cst)
    trace = os.environ.get("KERNEL_TRACE", "0") == "1"
    LAST = run_bass_kernel_spmd(nc, in_maps, core_ids=list(range(NCORES)),
                                trace=trace)
    # host gather: per-(b,g) partial sums * |s_a|, mask sums + gate on host
    S_a = np.zeros(3, np.float64)
    for k, r in enumerate(LAST.results):
        o = r["out"].astype(np.float64)                  # [128, 3]
        st = o.reshape(BPC, G, 3).sum(axis=1)            # [BPC, 3]
        S_a += (st * sabs[k * BPC:(k + 1) * BPC]).sum(axis=0)
    M_a = M.reshape(B, 3, HW).sum(axis=(0, 2)).astype(np.float64)  # exact ints
    loss = sum(0.0 if M_a[a] < 3 * B else S_a[a] for a in range(3))
    total = max(M_a.sum(), 1.0)
    return np.asarray(np.float32(loss / total))


# revision 9
# speedup vs baseline: 1.2678x; 1.2678x over previous
"""Trainium2 Bass kernel for nn_CT_loss (data-parallel over batch, 8 cores).

Math (R is a general 3x3 matrix, not orthogonal):
  u   = A P0 + b0          A = R diag(e), b0 = t - 0.5 R e      (per batch)
  c_a = G_a P0 + g0_a      G = R^T A,     g0 = R^T b0
  v~_ai = alpha q_A + beta q_B + h~      (1/s_a folded into alpha/beta/h~;
           host re-scales per-batch partial sums by |s_a| during the gather)
  d_ai = u_i - c_a v~_ai ;  la = sqrt(sum_i d^2 * m_a)
  loss = sum_a [sum(m_a) >= 3B] sum(la) / max(sum(m), 1)   (host gather)

Device design (v5): pixel-major [128 partitions = (b,g), free = 1024 px].
Measured TRN2 prices: DVE ts[1u] ~480ns, tt[1u] ~685ns, tt[3u] ~1750ns;
ACT term[1u] ~1225ns, square[3u] ~2860ns. GPSIMD unusable (slow ucode +
SBUF port contention with DVE). Split for ~36us/engine:
  - DVE : 21 terms (ts 4x: u/c T0+T2, VB) + all tensor_tensor work
  - ACT : 15 terms (u/c T1, VA), squares, sqrt+accum
DMA: x split per channel and issued first so T0 terms start ~9.5us;
q split per a-pair on the scalar ring; masks last.
Mask sums + gating + normalization on host (exact in f64).
"""
import os
import sys

import numpy as np

for _p in ("/opt/trn_rl_repo",):
    if _p not in sys.path:
        sys.path.insert(0, _p)

import concourse.bass as bass
import concourse.bacc as bacc
import concourse.tile as tile
from concourse import mybir
from concourse.bass_utils import run_bass_kernel_spmd

from ml_dtypes import bfloat16

F32 = mybir.dt.float32
BF16 = mybir.dt.bfloat16
AF = mybir.ActivationFunctionType
OP = mybir.AluOpType

B, HW = 64, 128 * 128
NCORES, BPC, G, F = 8, 8, 16, 1024

# a -> (c1, c2, qchA, qchB)
QCH = {0: (1, 2, 0, 1), 1: (0, 2, 2, 3), 2: (0, 1, 4, 5)}

# constants tile columns
CA = 0     # A[i*3+j] 9         (u coefs)
CB0 = 9    # b0[i] 3            (u bias)
CG = 12    # G[a*3+j] 9         (c coefs)
CG0 = 21   # g0[a] 3            (c bias)
CAL = 24   # alpha~[a*3+i] 9    (v qA coef)
CH = 33    # h~[a*3+i] 9        (v bias)
CBE = 42   # beta~[a*3+i] 9     (v qB coef)
CZ = 51    # 0.0
NCST = 52

_BUILT = None
LAST = None


def _bcast3(ap, n):
    """[128, F] AP -> [128, n, F] with step-0 middle dim."""
    return bass.AP(tensor=ap.tensor, offset=ap.offset,
                   ap=[ap.ap[0], [0, n], *ap.ap[1:]])


def _build_nc():
    nc = bacc.Bacc(None)
    x_d = nc.dram_tensor("x", [BPC, G, 3, F], BF16, kind="ExternalInput")
    q_d = nc.dram_tensor("q", [BPC, G, 6, F], BF16, kind="ExternalInput")
    m_d = nc.dram_tensor("m", [BPC, G, 3, F], BF16, kind="ExternalInput")
    c_d = nc.dram_tensor("cst", [128, NCST], F32, kind="ExternalInput")
    o_d = nc.dram_tensor("out", [128, 3], F32, kind="ExternalOutput")

    with tile.TileContext(nc) as tc:
        with tc.tile_pool(name="main", bufs=1) as pool:
            cst = pool.tile([128, NCST], F32, tag="cst")
            X = pool.tile([128, 3, F], BF16, tag="X")
            Q = pool.tile([128, 6, F], BF16, tag="Q")
            M = pool.tile([128, 3, F], BF16, tag="M")

            # DMA order matters: cst (tiny) + x per channel first on sync
            # (T0/T2 terms need only x0/x2); q per a-pair on scalar ring;
            # masks last (needed ~30us in).
            nc.sync.dma_start(cst[:], c_d[:])
            xr = x_d[:].rearrange("b g c f -> (b g) c f")
            for ch in range(3):
                nc.sync.dma_start(X[:, ch, :], xr[:, ch, :])
            qr = q_d[:].rearrange("b g c f -> (b g) c f")
            for a in range(3):
                nc.scalar.dma_start(Q[:, 2 * a:2 * a + 2, :],
                                    qr[:, 2 * a:2 * a + 2, :])
            nc.sync.dma_start(M[:], m_d[:].rearrange("b g c f -> (b g) c f"))

            def cs(j):
                return cst[:, j:j + 1]

            # warm the sqrt_and_others table (identity/square/sqrt co-reside)
            warm = pool.tile([128, 1], BF16, tag="warm")
            nc.scalar.activation(warm[:], cs(CZ), AF.Sqrt)

            acc = pool.tile([128, 3], F32, tag="acc")

            # term helper: out = coef*src + bias on the given engine
            def term(eng, out, src, coef, bias):
                if eng == "act":
                    nc.scalar.activation(out, src, AF.Identity,
                                         bias=bias if bias is not None else cs(CZ),
                                         scale=coef)
                else:
                    if bias is None:
                        nc.vector.tensor_scalar(out, src, coef, None,
                                                op0=OP.mult)
                    else:
                        nc.vector.tensor_scalar(out, src, coef, bias,
                                                op0=OP.mult, op1=OP.add)

            uc = pool.tile([128, 6, F], BF16, tag="uc")   # rows: u0..u2,c0..c2
            tA = pool.tile([128, 6, F], BF16, tag="tA")
            tB = pool.tile([128, 6, F], BF16, tag="tB")

            # T0 terms (x0, with bias): DVE ts; T1 (x1): ACT; T2 (x2): DVE
            for i in range(3):
                term("dve", uc[:, i, :], X[:, 0, :], cs(CA + 3 * i), cs(CB0 + i))
                term("act", tA[:, i, :], X[:, 1, :], cs(CA + 3 * i + 1), None)
                term("dve", tB[:, i, :], X[:, 2, :], cs(CA + 3 * i + 2), None)
            for a in range(3):
                term("dve", uc[:, 3 + a, :], X[:, 0, :], cs(CG + 3 * a), cs(CG0 + a))
                term("act", tA[:, 3 + a, :], X[:, 1, :], cs(CG + 3 * a + 1), None)
                term("dve", tB[:, 3 + a, :], X[:, 2, :], cs(CG + 3 * a + 2), None)
            # packed adds: uc += tA ; uc += tB  (DVE, [6u] each)
            nc.vector.tensor_tensor(uc[:], uc[:], tA[:], op=OP.add)
            nc.vector.tensor_tensor(uc[:], uc[:], tB[:], op=OP.add)
            u = uc[:, 0:3, :]

            v = pool.tile([128, 9, F], BF16, tag="v")
            vb = pool.tile([128, 9, F], BF16, tag="vb")
            t9 = pool.tile([128, 9, F], BF16, tag="t9")
            sq = pool.tile([128, 9, F], BF16, tag="sq")
            la2 = pool.tile([128, 3, F], BF16, tag="la2")
            scr = pool.tile([128, F], BF16, tag="scr")

            # two late VB terms ride ACT (fills its tail dip; DVE is critical)
            vb_eng = ["dve"] * 7 + ["act", "act"]
            for a in range(3):
                _, _, qA, qB = QCH[a]
                for i in range(3):
                    r = 3 * a + i
                    term("act", v[:, r, :], Q[:, qA, :], cs(CAL + r), cs(CH + r))
                    term(vb_eng[r], vb[:, r, :], Q[:, qB, :], cs(CBE + r), None)
                sl = slice(3 * a, 3 * a + 3)
                # v = VA + VB (DVE)
                nc.vector.tensor_tensor(v[:, sl, :], v[:, sl, :],
                                        vb[:, sl, :], op=OP.add)
                # t = c_a * v (DVE, c broadcast over i)
                nc.vector.tensor_tensor(t9[:, sl, :],
                                        _bcast3(uc[:, 3 + a, :], 3),
                                        v[:, sl, :], op=OP.mult)
                # d = u - t (DVE, in place into t9)
                nc.vector.tensor_tensor(t9[:, sl, :], u, t9[:, sl, :],
                                        op=OP.subtract)
                # squares (ACT)
                nc.scalar.activation(sq[:, sl, :], t9[:, sl, :], AF.Square)
                # la2 = sq0 + sq1 + sq2 (DVE)
                nc.vector.tensor_tensor(la2[:, a, :], sq[:, 3 * a, :],
                                        sq[:, 3 * a + 1, :], op=OP.add)
                nc.vector.tensor_tensor(la2[:, a, :], la2[:, a, :],
                                        sq[:, 3 * a + 2, :], op=OP.add)
                # w = la2 * mask (DVE, in place)
                nc.vector.tensor_tensor(la2[:, a, :], la2[:, a, :],
                                        M[:, a, :], op=OP.mult)
                # la = sqrt(w), accumulate along free dim (ACT)
                nc.scalar.activation(scr[:], la2[:, a, :], AF.Sqrt,
                                     accum_out=acc[:, a:a + 1])

            nc.sync.dma_start(o_d[:], acc[:])

    nc.compile()
    return nc


def get_nc():
    global _BUILT
    if _BUILT is None:
        _BUILT = _build_nc()
    return _BUILT


def host_constants(R, T, E):
    """[B, NCST] fp32 constants (fp64 host math) + [B,3] |s| scales."""
    Bn = R.shape[0]
    out = np.zeros((Bn, NCST), np.float64)
    sabs = np.zeros((Bn, 3), np.float64)
    for b in range(Bn):
        Rb = R[b].astype(np.float64)
        tb = T[b].astype(np.float64)
        eb = E[b].astype(np.float64)
        A = Rb * eb[None, :]
        b0 = tb - 0.5 * (Rb @ eb)
        Gm = Rb.T @ A
        g0 = Rb.T @ b0
        s = Rb.T @ tb
        out[b, CA:CA + 9] = A.reshape(-1)
        out[b, CB0:CB0 + 3] = b0
        out[b, CG:CG + 9] = Gm.reshape(-1)
        out[b, CG0:CG0 + 3] = g0
        for a, (c1, c2, _, _) in QCH.items():
            sh = np.sign(s[a]) * max(abs(s[a]), 1e-12) if s[a] != 0 else 1e-12
            sabs[b, a] = abs(s[a])
            h = tb - 0.5 * (A[:, c1] + A[:, c2])
            out[b, CAL + 3 * a:CAL + 3 * a + 3] = A[:, c1] / sh
            out[b, CBE + 3 * a:CBE + 3 * a + 3] = A[:, c2] / sh
            out[b, CH + 3 * a:CH + 3 * a + 3] = h / sh
    return out.astype(np.float32), sabs


def make_in_maps(P0, Q0, M, cst):
    in_maps = []
    for k in range(NCORES):
        sl = slice(k * BPC, (k + 1) * BPC)
        in_maps.append({
            "x": P0[sl].reshape(BPC, 3, G, F).transpose(0, 2, 1, 3).astype(bfloat16),
            "q": Q0[sl].reshape(BPC, 6, G, F).transpose(0, 2, 1, 3).astype(bfloat16),
            "m": M[sl].reshape(BPC, 3, G, F).transpose(0, 2, 1, 3).astype(bfloat16),
            "cst": np.ascontiguousarray(np.repeat(cst[sl], G, axis=0)),
        })
    return in_maps


def kernel(pred_rots, pred_P0, pred_Q0, gt_occmask, roi_extent, pred_transes):
    global LAST
    R = np.asarray(pred_rots, np.float32)
    P0 = np.asarray(pred_P0, np.float32)
    Q0 = np.asarray(pred_Q0, np.float32)
    M = np.asarray(gt_occmask, np.float32)
    E = np.asarray(roi_extent, np.float32)
    T = np.asarray(pred_transes, np.float32)

    nc = get_nc()
    cst, sabs = host_constants(R, T, E)
    in_maps = make_in_maps(P0, Q0, M, cst)
    trace = os.environ.get("KERNEL_TRACE", "0") == "1"
    LAST = run_bass_kernel_spmd(nc, in_maps, core_ids=list(range(NCORES)),
                                trace=trace)
    # host gather: per-(b,g) partial sums * |s_a|, mask sums + gate on host
    S_a = np.zeros(3, np.float64)
    for k, r in enumerate(LAST.results):
        o = r["out"].astype(np.float64)                  # [128, 3]
        st = o.reshape(BPC, G, 3).sum(axis=1)            # [BPC, 3]
        S_a += (st * sabs[k * BPC:(k + 1) * BPC]).sum(axis=0)
    M_a = M.reshape(B, 3, HW).sum(axis=(0, 2)).astype(np.float64)  # exact ints
    loss = sum(0.0 if M_a[a] < 3 * B else S_a[a] for a in range(3))
    total = max(M_a.sum(), 1.0)
    return np.asarray(np.float32(loss / total))
